# revision 1
# baseline (speedup 1.0000x reference)
"""Trainium2 Bass kernel for nn_EnergyDistributionCNN (3x3 conv -> unfold ->
softmax over patch -> weighted -> fold overlap-add), 8 NeuronCores.

Math (algebraically identical to the torch/jax reference):
    out = conv3x3(x, k)            cross-correlation, zero pad 1
    E   = exp(out)
    Z   = boxsum3x3(E padded with ONES)   (zero pads contribute exp(0)=1)
    U   = x / Z
    S   = boxsum3x3(U zero-padded)
    result = E * S

Sharding: row-block across 8 cores with a 3-row halo sliced on the host
(zero-filled at the global edges) -- no device-to-device communication.
Global boundary rows are handled uniformly by a per-row mask fused into the
exp's per-partition scale (exp(0*out)=1); boundary columns by host zero
padding plus static edge memsets.

On-core layout: rows on partitions, cols on the free dim, processed in
row-tiles (<=122 output rows) x width-halves. All vertical stencil mixing
runs on the TensorEngine via banded matrices; horizontal mixing is 3
column-shifted matmuls accumulated in PSUM. Everything on the PE uses
fp32r (full-rate moving operand, ~11-bit mantissa); the conv -- whose
error exp() amplifies -- is error-compensated with a hi/lo split:
    conv = Mhi @ Xhi + Mhi @ Xlo + Mlo @ Xhi       (~fp32 quality)
where Xhi is the fp32r-rounded x (DVE copy) and Xlo = x - Xhi.
exp runs on the ScalarEngine directly from conv's PSUM; 1/Z uses the DVE
fast reciprocal (~18 bits). Band row-mappings put every compute op at
partition base 0; the valid output rows sit at partitions [2, R+2), which
the (partition-unrestricted) output DMA reads.
"""

from contextlib import ExitStack

import numpy as np

import concourse.bacc as bacc
import concourse.mybir as mybir
import concourse.tile as tile
from concourse._compat import with_exitstack
from concourse.bass_utils import run_bass_kernel_spmd

F32 = mybir.dt.float32
F32R = mybir.dt.float32r

H = 4096
W = 4096
N_CORES = 8
RC = H // N_CORES  # rows per core
HALO = 3
RT = 122   # output rows per row-tile (RT + 6 <= 128 partitions)
WS = 2     # width splits (SBUF capacity)
WH = W // WS
C = 512    # matmul column chunk = one fp32 PSUM bank
NBUFS = 3
PS_BUFS = 3


# ---------------------------------------------------------------- host side

def _make_bands(k: np.ndarray) -> np.ndarray:
    """bands[v][p, m] = k[p-m, v] (conv, v=0..2); bands[3] = BB ones with
    p-m in 0..2 (S matmul); bands[4] = BT ones with m-p in 0..2 (Z).
    bands[5..9]: same five patterns as 4x block-diagonal 32x32 blocks, for
    the column-folded last row-tile."""
    bands = np.zeros((10, 128, 128), np.float32)
    idx = np.arange(128)
    for d in range(3):
        p = idx[d:]
        m = idx[: 128 - d]
        for v in range(3):
            bands[v, p, m] = k[d, v]
        bands[3, p, m] = 1.0
        bands[4, m, p] = 1.0
    for i in range(5):
        blk = bands[i][:32, :32]
        for b in range(4):
            bands[5 + i][32 * b : 32 * b + 32, 32 * b : 32 * b + 32] = blk
    return bands


def _make_core_inputs(x: np.ndarray, bands: np.ndarray, core: int):
    r0 = core * RC
    lo, hi = r0 - HALO, r0 + RC + HALO
    # 26 extra zero rows let the folded last tile load full 32-row blocks
    xh = np.zeros((RC + 2 * HALO + 26, W + 2 * HALO), np.float32)
    s_lo, s_hi = max(lo, 0), min(hi, H)
    xh[s_lo - lo : s_hi - lo, HALO : HALO + W] = x[s_lo:s_hi]
    gl = np.arange(lo, hi)
    mask = ((gl >= 0) & (gl < H)).astype(np.float32)[:, None]
    return {"xh": xh, "mask": mask, "bands": bands}


def _make_tiles():
    tiles = []
    o = 0
    while o < RC:
        R = min(RT, RC - o)
        tiles.append((o, R))
        o += R
    return tiles


def _chunks(total: int):
    out = []
    s = 0
    while s < total:
        out.append((s, min(C, total - s)))
        s += C
    return out


# -------------------------------------------------------------- device side

@with_exitstack
def _energy_body(ctx: ExitStack, tc, out_d, xh_d, mask_d, bands_d):
    nc = tc.nc
    Exp = mybir.ActivationFunctionType.Exp

    # ---- constants: ONE DMA for all band matrices, hi/lo split on device;
    # the folded set is materialized first (the first emitted unit needs it)
    consts = ctx.enter_context(tc.tile_pool(name="consts", bufs=1))
    bigb = consts.tile([128, 10 * 128], F32, name="bigb")
    nc.sync.dma_start(
        out=bigb.rearrange("p (i m) -> p i m", i=10),
        in_=bands_d.rearrange("i p m -> p i m"),
    )

    def load_bands(base, suffix):
        mhi, mlo = [], []
        for v in range(3):
            mf = bigb[:, (base + v) * 128 : (base + v + 1) * 128]
            hi = consts.tile([128, 128], F32R, name=f"mhi{suffix}{v}")
            nc.vector.tensor_copy(out=hi, in_=mf)
            mhi.append(hi)
            lo = consts.tile([128, 128], F32R, name=f"mlo{suffix}{v}")
            nc.vector.tensor_sub(out=lo, in0=mf, in1=hi)
            mlo.append(lo)
        bb = consts.tile([128, 128], F32R, name=f"bb{suffix}")
        nc.vector.tensor_copy(out=bb, in_=bigb[:, (base + 3) * 128 : (base + 4) * 128])
        bt = consts.tile([128, 128], F32R, name=f"bt{suffix}")
        nc.vector.tensor_copy(out=bt, in_=bigb[:, (base + 4) * 128 : (base + 5) * 128])
        return mhi, mlo, bb, bt

    MhiF, MloF, BBF, BTF = load_bands(5, "f")
    Mhi, Mlo, BB, BT = load_bands(0, "")
    SEGW = WH // 4

    xpool = ctx.enter_context(tc.tile_pool(name="xp", bufs=NBUFS))
    xhip = ctx.enter_context(tc.tile_pool(name="xhip", bufs=NBUFS))
    xlop = ctx.enter_context(tc.tile_pool(name="xlop", bufs=NBUFS))
    epool = ctx.enter_context(tc.tile_pool(name="ep", bufs=NBUFS))
    upool = ctx.enter_context(tc.tile_pool(name="up", bufs=NBUFS))
    rzpool = ctx.enter_context(tc.tile_pool(name="rzp", bufs=3))
    respool = ctx.enter_context(tc.tile_pool(name="resp", bufs=NBUFS))
    mpool = ctx.enter_context(tc.tile_pool(name="mp", bufs=2))
    ps_conv = ctx.enter_context(tc.tile_pool(name="psc", bufs=PS_BUFS, space="PSUM"))
    ps_z = ctx.enter_context(tc.tile_pool(name="psz", bufs=2, space="PSUM"))
    ps_s = ctx.enter_context(tc.tile_pool(name="pss", bufs=2, space="PSUM"))

    tiles = _make_tiles()

    def fold_unit(o, R, h):
        # Column-folded last row-tile: 4 width-segments of one half stacked
        # on 32-partition blocks, block-diagonal bands, ops span all 128
        # partitions (off-band lanes hold finite junk; masked exp gives
        # E=1 and the extended Z band keeps Z>0 there).
        mk = mpool.tile([128, 1], F32, tag="mk")
        nc.vector.memset(mk, 0.0)
        for b in range(4):
            nc.sync.dma_start(
                out=mk[32 * b : 32 * b + R + 4], in_=mask_d[o + 1 : o + R + 5, :]
            )
        if True:
            if True:
                g0 = h * WH
                X = xpool.tile([128, WH + 6], F32, tag="X")
                for b in range(4):
                    nc.sync.dma_start(
                        out=X[32 * b : 32 * b + 32, : SEGW + 6],
                        in_=xh_d[o : o + 32, g0 + b * SEGW : g0 + b * SEGW + SEGW + 6],
                    )
                Xhi = xhip.tile([128, WH + 6], F32R, tag="Xhi")
                nc.vector.tensor_copy(out=Xhi[:, : SEGW + 6], in_=X[:, : SEGW + 6])
                Xlo = xlop.tile([128, WH + 6], F32R, tag="Xlo")
                nc.vector.tensor_sub(
                    out=Xlo[:, : SEGW + 6],
                    in0=X[:, : SEGW + 6],
                    in1=Xhi[:, : SEGW + 6],
                )

                E = epool.tile([128, WH + 4], F32R, tag="E")
                for cs, cl in _chunks(SEGW + 4):
                    pc = ps_conv.tile([128, C], F32, tag="pc")
                    mms = []
                    for v in range(3):
                        mms.append((MhiF[v], Xhi, v))
                        mms.append((MloF[v], Xhi, v))
                    for v in range(3):
                        mms.append((MhiF[v], Xlo, v))
                    for i, (mband, xop, v) in enumerate(mms):
                        nc.tensor.matmul(
                            pc[:, :cl],
                            mband,
                            xop[:, cs + v : cs + v + cl],
                            start=(i == 0),
                            stop=(i == len(mms) - 1),
                        )
                    nc.scalar.activation(E[:, cs : cs + cl], pc[:, :cl], Exp, scale=mk)
                if h == 0:
                    nc.vector.memset(E[0:32, 0:2].bitcast(F32), 1.0)
                if h == WS - 1:
                    nc.vector.memset(E[96:128, SEGW + 2 : SEGW + 4].bitcast(F32), 1.0)

                U = upool.tile([128, WH + 2], F32R, tag="U")
                for cs, cl in _chunks(SEGW + 2):
                    pz = ps_z.tile([128, C], F32, tag="pz")
                    for v in range(3):
                        nc.tensor.matmul(
                            pz[:, :cl],
                            BTF,
                            E[:, cs + v : cs + v + cl],
                            start=(v == 0),
                            stop=(v == 2),
                        )
                    Rz = rzpool.tile([128, C], F32, tag="Rz")
                    nc.vector.reciprocal_approx_fast(out=Rz[:, :cl], in_=pz[:, :cl])
                    nc.vector.tensor_mul(
                        out=U[:, cs : cs + cl],
                        in0=X[:, cs + 2 : cs + 2 + cl],
                        in1=Rz[:, :cl],
                    )
                if h == 0:
                    nc.vector.memset(U[0:32, 0:1].bitcast(F32), 0.0)
                if h == WS - 1:
                    nc.vector.memset(U[96:128, SEGW + 1 : SEGW + 2].bitcast(F32), 0.0)

                res = respool.tile([128, WH], F32, tag="res")
                for cs, cl in _chunks(SEGW):
                    ps = ps_s.tile([128, C], F32, tag="ps")
                    for v in range(3):
                        nc.tensor.matmul(
                            ps[:, :cl],
                            BBF,
                            U[:, cs + v : cs + v + cl],
                            start=(v == 0),
                            stop=(v == 2),
                        )
                    nc.vector.tensor_mul(
                        out=res[:, cs : cs + cl],
                        in0=E[:, cs + 2 : cs + 2 + cl],
                        in1=ps[:, :cl],
                    )
                for b in range(4):
                    nc.sync.dma_start(
                        out=out_d[o : o + R, g0 + b * SEGW : g0 + (b + 1) * SEGW],
                        in_=res[32 * b + 2 : 32 * b + 2 + R, :SEGW],
                    )
            return

    def normal_tile(o, R):
        mk = mpool.tile([128, 1], F32, tag="mk")
        nc.sync.dma_start(out=mk[: R + 4], in_=mask_d[o + 1 : o + R + 5, :])
        for h in range(WS):
            g0 = h * WH
            # X[p, j] <-> (row r-3+p, global col g0-3+j)
            X = xpool.tile([128, WH + 6], F32, tag="X")
            nc.sync.dma_start(
                out=X[: R + 6, :], in_=xh_d[o : o + R + 6, g0 : g0 + WH + 6]
            )
            Xhi = xhip.tile([128, WH + 6], F32R, tag="Xhi")
            nc.vector.tensor_copy(out=Xhi[: R + 6, :], in_=X[: R + 6, :])
            Xlo = xlop.tile([128, WH + 6], F32R, tag="Xlo")
            nc.vector.tensor_sub(
                out=Xlo[: R + 6, :], in0=X[: R + 6, :], in1=Xhi[: R + 6, :]
            )

            # conv + exp -> E[m, e] <-> (row r-2+m, global col g0-2+e)
            E = epool.tile([128, WH + 4], F32R, tag="E")
            for cs, cl in _chunks(WH + 4):
                pc = ps_conv.tile([128, C], F32, tag="pc")
                mms = []
                for v in range(3):
                    mms.append((Mhi[v], Xhi, v))
                    mms.append((Mlo[v], Xhi, v))
                for v in range(3):
                    mms.append((Mhi[v], Xlo, v))
                for i, (mband, xop, v) in enumerate(mms):
                    nc.tensor.matmul(
                        pc[: R + 4, :cl],
                        mband[: R + 6, : R + 4],
                        xop[: R + 6, cs + v : cs + v + cl],
                        start=(i == 0),
                        stop=(i == len(mms) - 1),
                    )
                nc.scalar.activation(
                    E[: R + 4, cs : cs + cl],
                    pc[: R + 4, :cl],
                    Exp,
                    scale=mk[: R + 4],
                )
            # global-edge columns of E represent pad pixels: exp(0) = 1
            if h == 0:
                nc.vector.memset(E[: R + 4, 0:2].bitcast(F32), 1.0)
            if h == WS - 1:
                nc.vector.memset(E[: R + 4, WH + 2 : WH + 4].bitcast(F32), 1.0)

            # Z (vertical via BT, X frame) -> Rz -> U[m, z] (global col g0-1+z)
            U = upool.tile([128, WH + 2], F32R, tag="U")
            for cs, cl in _chunks(WH + 2):
                pz = ps_z.tile([128, C], F32, tag="pz")
                for v in range(3):
                    nc.tensor.matmul(
                        pz[: R + 4, :cl],
                        BT[: R + 4, : R + 4],
                        E[: R + 4, cs + v : cs + v + cl],
                        start=(v == 0),
                        stop=(v == 2),
                    )
                Rz = rzpool.tile([128, C], F32, tag="Rz")
                nc.vector.reciprocal_approx_fast(
                    out=Rz[: R + 4, :cl], in_=pz[: R + 4, :cl]
                )
                nc.vector.tensor_mul(
                    out=U[: R + 4, cs : cs + cl],
                    in0=X[: R + 4, cs + 2 : cs + 2 + cl],
                    in1=Rz[: R + 4, :cl],
                )
            # U at global-edge pad columns is 0 (fold drops OOB)
            if h == 0:
                nc.vector.memset(U[: R + 4, 0:1].bitcast(F32), 0.0)
            if h == WS - 1:
                nc.vector.memset(U[: R + 4, WH + 1 : WH + 2].bitcast(F32), 0.0)

            # S (vertical via BB, E frame) + res = E * S
            res = respool.tile([128, WH], F32, tag="res")
            for cs, cl in _chunks(WH):
                ps = ps_s.tile([128, C], F32, tag="ps")
                for v in range(3):
                    nc.tensor.matmul(
                        ps[: R + 2, :cl],
                        BB[: R + 4, : R + 2],
                        U[: R + 4, cs + v : cs + v + cl],
                        start=(v == 0),
                        stop=(v == 2),
                    )
                nc.vector.tensor_mul(
                    out=res[: R + 2, cs : cs + cl],
                    in0=E[: R + 2, cs + 2 : cs + 2 + cl],
                    in1=ps[: R + 2, :cl],
                )
            # valid output rows sit at partitions [2, R+2)
            nc.sync.dma_start(
                out=out_d[o : o + R, g0 : g0 + WH], in_=res[2 : R + 2, :WH]
            )

    of, Rf = tiles[-1]
    if len(tiles) > 1 and Rf <= 26:
        # cheap folded units at both pipeline edges: fast fill and drain
        fold_unit(of, Rf, 0)
        for o, R in tiles[:-1]:
            normal_tile(o, R)
        fold_unit(of, Rf, WS - 1)
    else:
        for o, R in tiles:
            normal_tile(o, R)


_CACHE: dict = {}


def _build():
    if "nc" in _CACHE:
        return _CACHE["nc"]
    nc = bacc.Bacc(
        "TRN2", target_bir_lowering=False, debug=False, num_devices=N_CORES
    )
    xh_d = nc.dram_tensor(
        "xh", (RC + 2 * HALO + 26, W + 2 * HALO), F32, kind="ExternalInput"
    ).ap()
    mask_d = nc.dram_tensor("mask", (RC + 2 * HALO, 1), F32, kind="ExternalInput").ap()
    bands_d = nc.dram_tensor("bands", (10, 128, 128), F32, kind="ExternalInput").ap()
    out_d = nc.dram_tensor("out", (RC, W), F32, kind="ExternalOutput").ap()
    with tile.TileContext(nc) as tc:
        _energy_body(tc, out_d, xh_d, mask_d, bands_d)
    nc.compile()
    _CACHE["nc"] = nc
    return nc


def kernel(shareable_energy: np.ndarray, kernel: np.ndarray, **_run_kw) -> np.ndarray:
    x = np.ascontiguousarray(np.asarray(shareable_energy, np.float32))
    k = np.asarray(kernel, np.float32)
    assert x.shape == (H, W), x.shape
    nc = _build()
    bands = _make_bands(k)
    in_maps = [_make_core_inputs(x, bands, core) for core in range(N_CORES)]
    r = run_bass_kernel_spmd(nc, in_maps, core_ids=list(range(N_CORES)), **_run_kw)
    out = np.concatenate([res["out"] for res in r.results], axis=0)
    if _run_kw:
        _CACHE["last_result"] = r
    return out



# revision 59
# speedup vs baseline: 1.5389x; 1.5389x over previous
"""Trainium2 Bass kernel for nn_EnergyDistributionCNN (3x3 conv -> unfold ->
softmax over patch -> weighted -> fold overlap-add), 8 NeuronCores.

Math (algebraically identical to the torch/jax reference):
    out = conv3x3(x, k)            cross-correlation, zero pad 1
    E   = exp(out)
    Z   = boxsum3x3(E padded with ONES)   (zero pads contribute exp(0)=1)
    U   = x / Z
    S   = boxsum3x3(U zero-padded)
    result = E * S

Sharding: row-block across 8 cores with a 3-row halo sliced on the host
(zero-filled at the global edges) -- no device-to-device communication.

Engine split (per width-half row-tile unit, ~2050 cols):
  PE (7 full-width passes): conv as 3 shifted banded matmuls (fp32r, no
    hi/lo compensation -- rel-err budget allows it), Z = 3 shifted
    all-ones band passes on E (fp32r), S-vertical = 1 all-ones band pass
    on the horizontally pre-summed U (bf16).
  Scalar: exp (masked via per-partition scale) and Copy (PSUM -> SBUF
    drain of Z). Copy shares exp's activation table -- Reciprocal does
    not, and mixing them would thrash 1283ns table loads.
  GpSimd:  U = x / Z  (SBUF-only operands; gpsimd has no PSUM port).
  DVE:     2 bf16 horizontal-shift adds building hor3(U), and the final
    res = E * S multiply from PSUM.
Stages are software-pipelined with a one-unit lag (S/mulres/store of unit
i-1 are emitted after the conv/Z chain of unit i) so no engine waits on
the cross-engine dependency chain of the unit it is currently processing.
"""

from contextlib import ExitStack

import numpy as np

import concourse.bacc as bacc
import concourse.mybir as mybir
import concourse.tile as tile
from concourse._compat import with_exitstack
from concourse.bass_utils import run_bass_kernel_spmd

F32 = mybir.dt.float32
F32R = mybir.dt.float32r
BF16 = mybir.dt.bfloat16

H = 4096
W = 4096
N_CORES = 8
RC = H // N_CORES  # rows per core
HALO = 3
RT = 122   # output rows per row-tile (RT + 6 <= 128 partitions)
WS = 2     # width splits (SBUF capacity)
WH = W // WS
MM = 512   # matmul moving-operand max free size / one fp32 PSUM bank
NBUFS = 3


# ---------------------------------------------------------------- host side

def _make_bands(k: np.ndarray) -> np.ndarray:
    """bands[v][p, m] = k[p-m, v] (conv, v=0..2); bands[3] = BB ones with
    p-m in 0..2 (S matmul); bands[4] = BT ones with m-p in 0..2 (Z).
    bands[5..9]: same five patterns as 4x block-diagonal 32x32 blocks, for
    the column-folded last row-tile."""
    bands = np.zeros((10, 128, 128), np.float32)
    idx = np.arange(128)
    for d in range(3):
        p = idx[d:]
        m = idx[: 128 - d]
        for v in range(3):
            bands[v, p, m] = k[d, v]
        bands[3, p, m] = 1.0
        bands[4, m, p] = 1.0
    for i in range(5):
        blk = bands[i][:32, :32]
        for b in range(4):
            bands[5 + i][32 * b : 32 * b + 32, 32 * b : 32 * b + 32] = blk
    return bands


def _make_core_inputs(x: np.ndarray, bands: np.ndarray, core: int):
    r0 = core * RC
    lo, hi = r0 - HALO, r0 + RC + HALO
    # 26 extra zero rows let the folded last tile load full 32-row blocks
    xh = np.zeros((RC + 2 * HALO + 26, W + 2 * HALO), np.float32)
    s_lo, s_hi = max(lo, 0), min(hi, H)
    xh[s_lo - lo : s_hi - lo, HALO : HALO + W] = x[s_lo:s_hi]
    gl = np.arange(lo, hi)
    mask = ((gl >= 0) & (gl < H)).astype(np.float32)
    # pre-tiled per-row-tile mask: column j = exp-scale rows for tile j
    # (rows o+1 .. o+R+4); the fold tile's column is laid out in its
    # 4x32-partition block structure with zeros on the unused lanes.
    tiles = _make_tiles()
    mk = np.zeros((128, len(tiles)), np.float32)
    for j, (o, R) in enumerate(tiles[:-1]):
        mk[: R + 4, j] = mask[o + 1 : o + R + 5]
    of, Rf = tiles[-1]
    if Rf <= 26:
        for b in range(4):
            mk[32 * b : 32 * b + Rf + 4, len(tiles) - 1] = mask[of + 1 : of + Rf + 5]
    else:
        mk[: Rf + 4, len(tiles) - 1] = mask[of + 1 : of + Rf + 5]
    # fold tile's X pre-packed in its 4x32-partition block layout (one DMA
    # per width-half instead of four)
    SEGW = WH // 4
    xf = np.zeros((2, 128, SEGW + 6), np.float32)
    for j, h in enumerate((0, WS - 1)):
        g0 = h * WH
        for b in range(4):
            xf[j, 32 * b : 32 * b + 32, :] = xh[
                of : of + 32, g0 + b * SEGW : g0 + b * SEGW + SEGW + 6
            ]
    return {"xh": xh, "mask": mk, "bands": bands, "xf": xf}


def _make_tiles():
    tiles = []
    o = 0
    while o < RC:
        R = min(RT, RC - o)
        tiles.append((o, R))
        o += R
    return tiles


def _chunks(total: int, step: int = MM):
    out = []
    s = 0
    while s < total:
        out.append((s, min(step, total - s)))
        s += step
    return out


# -------------------------------------------------------------- device side

@with_exitstack
def _energy_body(ctx: ExitStack, tc, out_d, xh_d, mask_d, bands_d, xf_d):
    nc = tc.nc
    Exp = mybir.ActivationFunctionType.Exp
    Cpy = mybir.ActivationFunctionType.Copy

    # ---- constants. Conv/BT bands are used directly as fp32r bitcast
    # views; BB (all ones) additionally as bf16. The folded-band half is
    # DMA'd first: the first emitted unit (the folded row-tile) needs it.
    consts = ctx.enter_context(tc.tile_pool(name="consts", bufs=1))
    scratch = consts.tile([1, 2], F32, name="scratch")
    nc.vector.memset(scratch, 0.0)
    # dummy activation at t=0 hoists the 1283ns Exp-table load off the
    # critical path (it would otherwise sit behind the first exp's waits)
    nc.scalar.activation(scratch[:, 0:1], scratch[:, 1:2],
                         mybir.ActivationFunctionType.Exp)
    bigb = consts.tile([128, 10 * 128], F32R, name="bigb")
    nc.sync.dma_start(
        out=bigb[:, 5 * 128 :].rearrange("p (i m) -> p i m", i=5),
        in_=bands_d[5:].rearrange("i p m -> p i m"),
    )

    def band(i):
        return bigb[:, i * 128 : (i + 1) * 128]

    MB = [band(v) for v in range(3)]       # conv bands, normal
    BT = band(4)                            # Z band, normal
    MBF = [band(5 + v) for v in range(3)]  # conv bands, folded
    BTF = band(9)                           # Z band, folded
    BBFb = consts.tile([128, 128], BF16, name="bbfb")
    nc.vector.tensor_copy(out=BBFb, in_=bigb[:, 8 * 128 : 9 * 128].bitcast(F32))
    BBb = consts.tile([128, 128], BF16, name="bbb")

    def load_normal_bands():
        # deferred until after the first (folded) unit's X DMAs so the
        # pipeline-fill unit's inputs are first in the DMA queue
        nc.sync.dma_start(
            out=bigb[:, : 5 * 128].rearrange("p (i m) -> p i m", i=5),
            in_=bands_d[:5].rearrange("i p m -> p i m"),
        )
        nc.vector.tensor_copy(out=BBb, in_=bigb[:, 3 * 128 : 4 * 128].bitcast(F32))

    SEGW = WH // 4
    tiles = _make_tiles()

    # all row-tile exp-scale masks arrive in one small DMA (host pre-tiled)
    mk_all = consts.tile([128, len(tiles)], F32, name="mk_all")
    nc.sync.dma_start(out=mk_all, in_=mask_d)

    xpool = ctx.enter_context(tc.tile_pool(name="xp", bufs=6))
    epool = ctx.enter_context(tc.tile_pool(name="ep", bufs=4))
    zspool = ctx.enter_context(tc.tile_pool(name="zsp", bufs=2))
    rzpool = ctx.enter_context(tc.tile_pool(name="rzp", bufs=NBUFS))
    upool = ctx.enter_context(tc.tile_pool(name="up", bufs=NBUFS))
    tpool = ctx.enter_context(tc.tile_pool(name="tp", bufs=NBUFS))
    uhpool = ctx.enter_context(tc.tile_pool(name="uhp", bufs=2))
    respool = ctx.enter_context(tc.tile_pool(name="resp", bufs=4))
    ps_conv = ctx.enter_context(tc.tile_pool(name="psc", bufs=2, space="PSUM"))
    ps_z = ctx.enter_context(tc.tile_pool(name="psz", bufs=2, space="PSUM"))
    ps_s = ctx.enter_context(tc.tile_pool(name="pss", bufs=2, space="PSUM"))

    def stage_a1(unit):
        """X load -> conv (PE) -> exp (Scalar) -> Z (PE) -> copy (Scalar).
        Returns unit state."""
        o, R, g0, cw, fold, tj = unit
        EW = (SEGW if fold else cw) + 4   # E width
        UW = EW - 2                       # U / Rz width

        mk = mk_all[:, tj : tj + 1]

        # X[p, j] <-> (row r-3+p, global col g0-3+j). The xh dram tensor is
        # declared float32r so the PE can consume the DMA'd tile directly
        # (the BIR verifier requires fp32r matmul operands to be produced
        # as fp32r; a DMA preserves the declared-rounded dtype).
        X = xpool.tile([128, EW + 2], F32R, tag="X")
        if fold:
            nc.sync.dma_start(out=X, in_=xf_d[0 if g0 == 0 else 1])
        else:
            nc.sync.dma_start(
                out=X[: R + 6, :], in_=xh_d[o : o + R + 6, g0 : g0 + cw + 6]
            )
        Xr = X
        mb = MBF if fold else MB
        bt = BTF if fold else BT
        bb = BBFb if fold else BBb
        rows_in = slice(0, 128) if fold else slice(0, R + 6)
        rows_e = slice(0, 128) if fold else slice(0, R + 4)

        # conv + exp -> E[m, e] <-> (row r-2+m, global col g0-2+e).
        # PSUM tiles are 1024 wide (2 banks); matmuls fill 512-wide
        # accumulation groups, exp reads the whole tile at once.
        E = epool.tile([128, EW], F32R, tag="E")
        for cs, cl in _chunks(EW, 2 * MM):
            pc = ps_conv.tile([128, 2 * MM], F32, tag="pc")
            for ss, sl in _chunks(cl):
                for v in range(3):
                    nc.tensor.matmul(
                        pc[rows_e, ss : ss + sl],
                        mb[v][rows_in, rows_e],
                        Xr[rows_in, cs + ss + v : cs + ss + v + sl],
                        start=(v == 0),
                        stop=(v == 2),
                    )
            nc.scalar.activation(
                E[rows_e, cs : cs + cl], pc[rows_e, :cl], Exp,
                scale=mk if fold else mk[: R + 4],
            )
        # E at global-edge pad columns must be exp(0)=1: the conv window
        # at pad col -1 / W overlaps one real column, so it is NOT zero
        if g0 == 0:
            er = slice(0, 32) if fold else rows_e
            nc.vector.memset(E[er, 0:2].bitcast(F32), 1.0)
        if g0 + cw == W:
            er = slice(96, 128) if fold else rows_e
            nc.vector.memset(E[er, EW - 2 : EW].bitcast(F32), 1.0)

        # Z (3 shifted BT passes on E) -> drain PSUM to SBUF (Scalar Copy)
        Zs = zspool.tile([128, UW], F32, tag="Zs")
        for cs, cl in _chunks(UW):
            pz = ps_z.tile([128, MM], F32, tag="pz")
            for v in range(3):
                nc.tensor.matmul(
                    pz[rows_e, :cl],
                    bt[rows_e, rows_e],
                    E[rows_e, cs + v : cs + v + cl],
                    start=(v == 0),
                    stop=(v == 2),
                )
            nc.scalar.activation(Zs[rows_e, cs : cs + cl], pz[rows_e, :cl], Cpy)

        return dict(o=o, R=R, g0=g0, cw=cw, fold=fold, UW=UW, X=X, E=E,
                    Zs=Zs, bb=bb, rows_e=rows_e, uh_dve=False)

    def stage_a2(st):
        """Rz = 1/Z (DVE approx-fast, ~51 ULP) -> U = x * Rz (GpSimd, bf16).
        Emitted one unit behind stage_a1; the DVE recip's input (Zs) is then
        already complete, so it never parks at the DVE queue head."""
        rows_e, UW, X, Zs = st["rows_e"], st["UW"], st["X"], st["Zs"]
        Rz = rzpool.tile([128, UW], F32, tag="Rz")
        U = upool.tile([128, UW], BF16, tag="U")
        sp = min(UW, 2 * MM + 2)  # half split point
        st["halves"] = [(0, UW)] if sp >= UW else [(0, sp), (sp, UW)]
        for a, b in st["halves"]:
            nc.vector.reciprocal_approx_fast(
                out=Rz[rows_e, a:b], in_=Zs[rows_e, a:b]
            )
            nc.gpsimd.tensor_mul(
                out=U[rows_e, a:b], in0=X.bitcast(F32)[rows_e, 2 + a : 2 + b],
                in1=Rz[rows_e, a:b],
            )
        st["U"] = U

    def stage_a3(st):
        """Horizontal adds (DVE bf16 2x mode). All units compute
        t = U + U(shift 1); units flagged uh_dve also fold in the second
        shift (Uh = t + U(shift 2)), trading a DVE add for stage_b's
        second PE pass -- the flag ratio balances PE vs DVE load. Emitted
        after other DVE work so GpSimd's U multiply has had time to
        finish."""
        rows_e, UW, U = st["rows_e"], st["UW"], st["U"]
        t = tpool.tile([128, UW - 1], BF16, tag="t")
        for a, b in st["halves"]:
            # t[c] = U[c] + U[c+1] for c in [a, b-1); the second half also
            # covers the straddling element c = sp-1
            ta, tb = (max(0, a - 1), b - 1)
            nc.vector.tensor_add(
                out=t[rows_e, ta:tb], in0=U[rows_e, ta:tb],
                in1=U[rows_e, ta + 1 : tb + 1],
            )
        st["t"] = t
        if st["uh_dve"]:
            Uh = uhpool.tile([128, UW - 2], BF16, tag="Uh")
            for a, b in st["halves"]:
                ua, ub = (max(0, a - 2), b - 2)
                nc.vector.tensor_add(
                    out=Uh[rows_e, ua:ub], in0=t[rows_e, ua:ub],
                    in1=U[rows_e, ua + 2 : ub + 2],
                )
            st["Uh"] = Uh

    def stage_b(st):
        """S vertical pass(es) (PE, bf16) -> res = E * S (DVE) -> store.
        uh_dve units: one pass on the pre-summed Uh; others: BB@t +
        BB@U(shift 2) accumulate the box sum in two passes."""
        o, R, g0, cw, fold, E, t, U, bb, rows_e = (
            st["o"], st["R"], st["g0"], st["cw"], st["fold"], st["E"],
            st["t"], st["U"], st["bb"], st["rows_e"])
        OW = SEGW if fold else cw
        rows_s = slice(0, 128) if fold else slice(0, R + 2)
        res = respool.tile([128, OW], F32, tag="res")
        for cs, cl in _chunks(OW):
            ps = ps_s.tile([128, MM], F32, tag="ps")
            if st["uh_dve"]:
                nc.tensor.matmul(
                    ps[rows_s, :cl], bb[rows_e, rows_s],
                    st["Uh"][rows_e, cs : cs + cl],
                    start=True, stop=True,
                )
            else:
                nc.tensor.matmul(
                    ps[rows_s, :cl], bb[rows_e, rows_s], t[rows_e, cs : cs + cl],
                    start=True, stop=False,
                )
                nc.tensor.matmul(
                    ps[rows_s, :cl], bb[rows_e, rows_s],
                    U[rows_e, cs + 2 : cs + 2 + cl],
                    start=False, stop=True,
                )
            nc.vector.tensor_mul(
                out=res[rows_s, cs : cs + cl],
                in0=E[rows_s, cs + 2 : cs + 2 + cl],
                in1=ps[rows_s, :cl],
            )
        st["res"] = res

    def store(st):
        # lagged one unit behind stage_b so SP's out-DMA issue never waits
        # on an unfinished mulres (which would head-of-line-block the next
        # X prefetch in the queue)
        o, R, g0, cw, fold, res = (
            st["o"], st["R"], st["g0"], st["cw"], st["fold"], st["res"])
        if fold:
            nc.sync.dma_start(
                out=out_d[o : o + R, g0 : g0 + WH].rearrange(
                    "r (b c) -> b r c", b=4
                ),
                in_=res.rearrange("(b p) c -> b p c", b=4)[:, 2 : 2 + R, :SEGW],
            )
        else:
            nc.sync.dma_start(
                out=out_d[o : o + R, g0 : g0 + cw], in_=res[2 : R + 2, :cw]
            )

    of, Rf = tiles[-1]
    units = []
    if len(tiles) > 1 and Rf <= 26:
        # cheap folded units at both pipeline edges: fast fill and drain
        units.append((of, Rf, 0, WH, True, len(tiles) - 1))
        for j, (o, R) in enumerate(tiles[:-1]):
            for h in range(WS):
                units.append((o, R, h * WH, WH, False, j))
        # split the trailing normal unit in two: at drain time only the
        # cheap fold remains to hide a unit's cross-engine chain, and a
        # half-width unit's chain is half as long
        o, R, g0, cw, fold, j = units.pop()
        units.append((o, R, g0, cw // 2, fold, j))
        units.append((o, R, g0 + cw // 2, cw // 2, fold, j))
        units.append((of, Rf, (WS - 1) * WH, WH, True, len(tiles) - 1))
    else:
        for j, (o, R) in enumerate(tiles):
            for h in range(WS):
                units.append((o, R, h * WH, WH, False, j))

    # Software pipeline with per-engine queue discipline: each stage is
    # emitted only when its cross-engine inputs had a full iteration to
    # complete, and DVE work is interleaved (recip, mulres, adds) so no op
    # parks at the DVE queue head waiting on Scalar/GpSimd.
    states = [None] * len(units)
    n = len(units)
    # units whose second hor-add runs on DVE (instead of a second S pass
    # on PE) -- the ratio balances the two engines' steady-state load
    uh_dve_units: set = set()
    for i in range(n + 4):
        if i < n:
            states[i] = stage_a1(units[i])
            states[i]["uh_dve"] = i in uh_dve_units
            if i == 0:
                load_normal_bands()
        if 1 <= i <= n:
            stage_a2(states[i - 1])
        if 2 <= i <= n + 1:
            stage_b(states[i - 2])
        if 1 <= i <= n:
            stage_a3(states[i - 1])
        if 4 <= i <= n + 3:
            store(states[i - 4])


_CACHE: dict = {}


def _build():
    if "nc" in _CACHE:
        return _CACHE["nc"]
    nc = bacc.Bacc(
        "TRN2", target_bir_lowering=False, debug=False, num_devices=N_CORES
    )
    xh_d = nc.dram_tensor(
        "xh", (RC + 2 * HALO + 26, W + 2 * HALO), F32R, kind="ExternalInput"
    ).ap()
    mask_d = nc.dram_tensor(
        "mask", (128, len(_make_tiles())), F32, kind="ExternalInput"
    ).ap()
    bands_d = nc.dram_tensor("bands", (10, 128, 128), F32R, kind="ExternalInput").ap()
    xf_d = nc.dram_tensor(
        "xf", (2, 128, W // WS // 4 + 6), F32R, kind="ExternalInput"
    ).ap()
    out_d = nc.dram_tensor("out", (RC, W), F32, kind="ExternalOutput").ap()
    with tile.TileContext(nc) as tc:
        _energy_body(tc, out_d, xh_d, mask_d, bands_d, xf_d)
    nc.compile()
    _CACHE["nc"] = nc
    return nc


def kernel(shareable_energy: np.ndarray, kernel: np.ndarray, **_run_kw) -> np.ndarray:
    x = np.ascontiguousarray(np.asarray(shareable_energy, np.float32))
    k = np.asarray(kernel, np.float32)
    assert x.shape == (H, W), x.shape
    nc = _build()
    bands = _make_bands(k)
    in_maps = [_make_core_inputs(x, bands, core) for core in range(N_CORES)]
    r = run_bass_kernel_spmd(nc, in_maps, core_ids=list(range(N_CORES)), **_run_kw)
    out = np.concatenate([res["out"] for res in r.results], axis=0)
    if _run_kw:
        _CACHE["last_result"] = r
    return out
